# revision 1
# baseline (speedup 1.0000x reference)
"""CPAttention Trainium2 kernel: 8-way batch-data-parallel over 8 NeuronCores.

Per core (one batch element):
  qkT  = (x @ w_qkv[:, :1024]).T            fp32   [1024(c), 1024(i)]
  v    = x @ w_qkv[:, 1024:]                bf16   [1024(j), 512(d)]  (natural)
  per head h, per key-tile jt:
    dotsT[j, i] = k_h^T q_h                 fp32 matmul (K=64)
    t    = dotsT * maskT                    DVE  (fp32, mask bf16 exact 0/1)
    e    = exp(scale * t)                   ACT -> bf16
    ab   = |t|                              DVE  (sign-bit AND)
    oT  += [v_h | 1]^T e                    bf16 matmul -> [65, i] (row 64 = Z)
    sc  += ones^T ab                        fp32 matmul -> [1, i] (over h, jt)
  onorm_h = oT[0:64] * (1/Z broadcast)      -> bf16
  y[i]  = sum_h onorm_h^T @ w_out_h + b_out
  score = sc * scale / nnz
Host: argsort(score) batch-reversed + 16-step row swap (commutes with w_out),
applied to y.
"""
import numpy as np

import concourse.bacc as bacc
import concourse.tile as tile
from concourse import mybir
from concourse.bass_utils import run_bass_kernel_spmd

F32 = mybir.dt.float32
BF16 = mybir.dt.bfloat16
U32 = mybir.dt.uint32
AOP = mybir.AluOpType
AFT = mybir.ActivationFunctionType

B, N, DIM = 8, 1024, 512
HEADS, DH = 8, 64
INNER = 512
SCALE = DH ** -0.5
PATCHES = 16

_cache = {}


def _build():
    nc = bacc.Bacc()
    xT = nc.declare_dram_parameter("xT", [DIM, N], F32, isOutput=False)
    xTbf = nc.declare_dram_parameter("xTbf", [DIM, N], BF16, isOutput=False)
    maskT = nc.declare_dram_parameter("maskT", [N, N], BF16, isOutput=False)
    wqk = nc.declare_dram_parameter("wqk", [DIM, 2 * INNER], F32, isOutput=False)
    wvbf = nc.declare_dram_parameter("wvbf", [DIM, INNER], BF16, isOutput=False)
    wobf = nc.declare_dram_parameter("wobf", [INNER, DIM], BF16, isOutput=False)
    bout = nc.declare_dram_parameter("bout", [1, DIM], F32, isOutput=False)
    y_out = nc.declare_dram_parameter("y", [N, DIM], F32, isOutput=True)
    sc_out = nc.declare_dram_parameter("score", [1, N], F32, isOutput=True)

    with tile.TileContext(nc) as tc:
        with tc.tile_pool(name="cst", bufs=1) as cst, \
             tc.tile_pool(name="wrk", bufs=3) as wrk, \
             tc.tile_pool(name="eph", bufs=2) as eph, \
             tc.tile_pool(name="pp", bufs=2, space="PSUM") as pp, \
             tc.tile_pool(name="pacc", bufs=1, space="PSUM") as pacc:

            # ---- loads ----
            xt = cst.tile([128, 4, N], F32)
            nc.sync.dma_start(out=xt, in_=xT[:, :].rearrange("(t p) i -> p t i", p=128))
            xtb = cst.tile([128, 4, N], BF16)
            nc.sync.dma_start(out=xtb, in_=xTbf[:, :].rearrange("(t p) i -> p t i", p=128))
            msk = cst.tile([128, 8, N], BF16)
            nc.sync.dma_start(out=msk, in_=maskT[:, :].rearrange("(t p) i -> p t i", p=128))
            wq = cst.tile([128, 4, 2 * INNER], F32)
            nc.sync.dma_start(out=wq, in_=wqk[:, :].rearrange("(t p) c -> p t c", p=128))
            wvb = cst.tile([128, 4, INNER], BF16)
            nc.sync.dma_start(out=wvb, in_=wvbf[:, :].rearrange("(t p) c -> p t c", p=128))
            wob = cst.tile([64, 8, DIM], BF16)
            nc.sync.dma_start(out=wob, in_=wobf[:, :].rearrange("(h d) e -> d h e", d=64))
            bb = cst.tile([128, DIM], F32)
            nc.sync.dma_start(out=bb, in_=bout[0:1, :].to_broadcast([128, DIM]))

            ones32 = cst.tile([128, 1], F32)
            nc.vector.memset(ones32, 1.0)
            onesb1 = cst.tile([1, 128], BF16)
            nc.vector.memset(onesb1, 1.0)
            onesb128 = cst.tile([128, 1], BF16)
            nc.vector.memset(onesb128, 1.0)

            qkT = cst.tile([128, 8, N], F32)
            vo = cst.tile([128, HEADS, 8, DH + 1], BF16)
            onorm = cst.tile([64, HEADS, N], BF16)
            y_sb = cst.tile([128, 8, DIM], F32)

            # ---- QKV q/k part (fp32) ----
            for ct in range(8):
                for ic in range(2):
                    pq = pp.tile([128, N], F32, tag="pp")
                    for kt in range(4):
                        nc.tensor.matmul(
                            pq[:, 0:512],
                            wq[:, kt, ct * 128:(ct + 1) * 128],
                            xt[:, kt, ic * 512:(ic + 1) * 512],
                            start=(kt == 0), stop=(kt == 3))
                    nc.vector.tensor_copy(qkT[:, ct, ic * 512:(ic + 1) * 512], pq[:, 0:512])

            # ---- V part (bf16), into vo with a trailing ones column ----
            for jt in range(8):
                pv = pp.tile([128, N], F32, tag="pp")
                for kt in range(4):
                    nc.tensor.matmul(
                        pv[:, 0:512],
                        xtb[:, kt, jt * 128:(jt + 1) * 128],
                        wvb[:, kt, :],
                        start=(kt == 0), stop=(kt == 3))
                nc.vector.tensor_copy(
                    vo[:, :, jt, 0:DH],
                    pv[:, 0:512].rearrange("p (h d) -> p h d", h=HEADS))
            nc.vector.memset(vo[:, :, :, DH:DH + 1], 1.0)

            # ---- attention ----
            sc_ps = pacc.tile([1, N], F32, tag="sc")
            for h in range(8):
                hp = (h % 2) * 64
                qt = h // 2
                kt_ = 4 + h // 2
                oT = pacc.tile([65, N], F32, tag="oT")
                for jt in range(8):
                    dots = pp.tile([128, N], F32, tag="pp")
                    for ic in range(2):
                        nc.tensor.matmul(
                            dots[:, ic * 512:(ic + 1) * 512],
                            qkT[hp:hp + 64, kt_, jt * 128:(jt + 1) * 128],
                            qkT[hp:hp + 64, qt, ic * 512:(ic + 1) * 512],
                            start=True, stop=True)
                    t = wrk.tile([128, N], F32, tag="t")
                    nc.vector.tensor_tensor(out=t, in0=dots, in1=msk[:, jt, :],
                                            op=AOP.mult)
                    e = wrk.tile([128, N], BF16, tag="e")
                    nc.scalar.activation(out=e, in_=t, func=AFT.Exp, scale=SCALE)
                    ab = wrk.tile([128, N], F32, tag="ab")
                    nc.vector.tensor_scalar(
                        out=ab.bitcast(U32), in0=t.bitcast(U32),
                        scalar1=0x7FFFFFFF, scalar2=None, op0=AOP.bitwise_and)
                    for ic in range(2):
                        nc.tensor.matmul(
                            oT[:, ic * 512:(ic + 1) * 512],
                            vo[:, h, jt, :],
                            e[:, ic * 512:(ic + 1) * 512],
                            start=(jt == 0), stop=(jt == 7))
                    for ic in range(2):
                        nc.tensor.matmul(
                            sc_ps[0:1, ic * 512:(ic + 1) * 512],
                            ones32,
                            ab[:, ic * 512:(ic + 1) * 512],
                            start=(h == 0 and jt == 0), stop=(h == 7 and jt == 7))
                # normalize: onorm_h = oT[0:64] * (1/Z) broadcast over partitions
                rz = eph.tile([1, N], F32, tag="rz")
                nc.vector.reciprocal(rz, oT[64:65, :])
                rzb = eph.tile([1, N], BF16, tag="rzb")
                nc.vector.tensor_copy(rzb, rz)
                rbc = pp.tile([128, N], F32, tag="pp")
                for ic in range(2):
                    nc.tensor.matmul(
                        rbc[:, ic * 512:(ic + 1) * 512],
                        onesb1,
                        rzb[0:1, ic * 512:(ic + 1) * 512],
                        start=True, stop=True)
                rbs = eph.tile([128, N], BF16, tag="rbs")
                nc.scalar.copy(rbs, rbc)
                nc.vector.tensor_tensor(out=onorm[:, h, :], in0=oT[0:64, :],
                                        in1=rbs[0:64, :], op=AOP.mult)

            # ---- output projection ----
            for it in range(8):
                yp = pp.tile([128, N], F32, tag="pp")
                for h in range(8):
                    nc.tensor.matmul(
                        yp[:, 0:512],
                        onorm[:, h, it * 128:(it + 1) * 128],
                        wob[:, h, :],
                        start=(h == 0), stop=(h == 7))
                nc.vector.tensor_tensor(out=y_sb[:, it, :], in0=yp[:, 0:512],
                                        in1=bb, op=AOP.add)

            # ---- score epilogue: nnz + scale ----
            nz = pacc.tile([1, N], F32, tag="oT")
            for jt in range(8):
                for ic in range(2):
                    nc.tensor.matmul(
                        nz[0:1, ic * 512:(ic + 1) * 512],
                        onesb128,
                        msk[:, jt, ic * 512:(ic + 1) * 512],
                        start=(jt == 0), stop=(jt == 7))
            rnz = eph.tile([1, N], F32, tag="rz")
            nc.vector.reciprocal(rnz, nz)
            sc_sb = eph.tile([1, N], F32, tag="scs")
            nc.vector.scalar_tensor_tensor(
                out=sc_sb, in0=sc_ps, scalar=SCALE, in1=rnz,
                op0=AOP.mult, op1=AOP.mult)

            # ---- outputs (one DMA each, distinct queues) ----
            nc.sync.dma_start(
                out=y_out[:, :].rearrange("(it p) e -> p it e", p=128), in_=y_sb)
            nc.gpsimd.dma_start(out=sc_out[:, :], in_=sc_sb)
    nc.finalize()
    return nc


def _get_nc():
    if "nc" not in _cache:
        _cache["nc"] = _build()
    return _cache["nc"]


def _run_device(inputs, trace=False):
    x = np.asarray(inputs["x"], np.float32)
    cp_mask = np.asarray(inputs["cp_mask"])
    w_qkv = np.asarray(inputs["w_qkv"], np.float32)
    w_out = np.asarray(inputs["w_out"], np.float32)
    b_out = np.asarray(inputs["b_out"], np.float32)

    bf = mybir.dt.np(BF16)
    maskT = np.ascontiguousarray(cp_mask.T).astype(bf)
    wqk = np.ascontiguousarray(w_qkv[:, :2 * INNER])
    wvbf = np.ascontiguousarray(w_qkv[:, 2 * INNER:]).astype(bf)
    wobf = np.ascontiguousarray(w_out).astype(bf)
    boutr = np.ascontiguousarray(b_out.reshape(1, DIM))

    in_maps = []
    for b in range(B):
        xTb = np.ascontiguousarray(x[b].T)
        in_maps.append({
            "xT": xTb,
            "xTbf": xTb.astype(bf),
            "maskT": maskT,
            "wqk": wqk,
            "wvbf": wvbf,
            "wobf": wobf,
            "bout": boutr,
        })

    nc = _get_nc()
    res = run_bass_kernel_spmd(nc, in_maps, core_ids=list(range(B)), trace=trace)
    y = np.stack([res.results[b]["y"] for b in range(B)])        # [B, N, DIM]
    score = np.stack([res.results[b]["score"][0] for b in range(B)])  # [B, N]
    return y, score, res


def _apply_swap(y, score, patches):
    # mirror the reference: idx = argsort(score, axis=-1)[::-1]  (batch reversed)
    idx = np.argsort(score, axis=-1, kind="stable")[::-1]
    out = y.copy()
    clone = y
    bi = np.arange(B)
    for i in range(1, patches + 1):
        ti = idx[:, i]
        out[bi, i] = clone[bi, ti]
        out[bi, ti] = clone[:, i]
    return out


def kernel(**inputs):
    patches = int(np.asarray(inputs["patches_in_core_nodes"]))
    y, score, _ = _run_device(inputs, trace=False)
    return _apply_swap(y, score, patches)


# revision 4
# speedup vs baseline: 1.2848x; 1.2848x over previous
"""CPAttention Trainium2 kernel: 8-way batch-data-parallel over 8 NeuronCores.

v2: head-pair processing with PE packing.
  - dots: fp32, two heads row-packed (K=64 at tile_position (0,0)/(64,0))
  - AV:   bf16, two heads col-packed into one [128,1024] PSUM (cols 0:64/64:128)
  - pack: 4-col-packed ones-matmuls -> score_A(row0, fp32), score_B(row32, fp32),
          Z_A(row64, bf16), Z_B(row96, bf16), accumulated over j-tiles
  - outproj: per-pair K=128 bf16
Score path (argsort-critical) stays fp32; softmax/output path is bf16.
Host applies the argsort + 16-step row swap (commutes with w_out).
"""
import numpy as np

import concourse.bacc as bacc
import concourse.tile as tile
from concourse import mybir
from concourse.bass_utils import run_bass_kernel_spmd

F32 = mybir.dt.float32
BF16 = mybir.dt.bfloat16
U32 = mybir.dt.uint32
AOP = mybir.AluOpType
AFT = mybir.ActivationFunctionType

B, N, DIM = 8, 1024, 512
HEADS, DH = 8, 64
INNER = 512
SCALE = DH ** -0.5

_cache = {}


def _build():
    nc = bacc.Bacc()
    xT = nc.declare_dram_parameter("xT", [DIM, N], F32, isOutput=False)
    xTbf = nc.declare_dram_parameter("xTbf", [DIM, N], BF16, isOutput=False)
    maskT = nc.declare_dram_parameter("maskT", [N, N], BF16, isOutput=False)
    wqk = nc.declare_dram_parameter("wqk", [DIM, 2 * INNER], F32, isOutput=False)
    wvbf = nc.declare_dram_parameter("wvbf", [DIM, INNER], BF16, isOutput=False)
    wobf = nc.declare_dram_parameter("wobf", [INNER, DIM], BF16, isOutput=False)
    bout = nc.declare_dram_parameter("bout", [1, DIM], F32, isOutput=False)
    y_out = nc.declare_dram_parameter("y", [N, DIM], F32, isOutput=True)
    sc_out = nc.declare_dram_parameter("score", [1, N], F32, isOutput=True)

    with tile.TileContext(nc) as tc:
        with tc.tile_pool(name="cst", bufs=1) as cst, \
             tc.tile_pool(name="wrk", bufs=3) as wrk, \
             tc.tile_pool(name="eph", bufs=2) as eph, \
             tc.tile_pool(name="ppA", bufs=1, space="PSUM") as ppA, \
             tc.tile_pool(name="ppB", bufs=1, space="PSUM") as ppB, \
             tc.tile_pool(name="poT", bufs=1, space="PSUM") as poT, \
             tc.tile_pool(name="ppk", bufs=1, space="PSUM") as ppk:

            # ---- loads ----
            xt = cst.tile([128, 4, N], F32)
            nc.sync.dma_start(out=xt, in_=xT[:, :].rearrange("(t p) i -> p t i", p=128))
            xtb = cst.tile([128, 4, N], BF16)
            nc.sync.dma_start(out=xtb, in_=xTbf[:, :].rearrange("(t p) i -> p t i", p=128))
            msk = cst.tile([128, 8, N], BF16)
            nc.sync.dma_start(out=msk, in_=maskT[:, :].rearrange("(t p) i -> p t i", p=128))
            wq = cst.tile([128, 4, 2 * INNER], F32)
            nc.sync.dma_start(out=wq, in_=wqk[:, :].rearrange("(t p) c -> p t c", p=128))
            wvb = cst.tile([128, 4, INNER], BF16)
            nc.sync.dma_start(out=wvb, in_=wvbf[:, :].rearrange("(t p) c -> p t c", p=128))
            wob = cst.tile([128, 4, DIM], BF16)
            nc.sync.dma_start(out=wob, in_=wobf[:, :].rearrange("(t p) e -> p t e", p=128))
            bb = cst.tile([128, DIM], F32)
            nc.sync.dma_start(out=bb, in_=bout[0:1, :].to_broadcast([128, DIM]))

            ones32 = cst.tile([128, 1], F32)
            nc.vector.memset(ones32, 1.0)
            onesbf = cst.tile([128, 1], BF16)
            nc.vector.memset(onesbf, 1.0)
            onesr1 = cst.tile([1, 128], BF16)
            nc.vector.memset(onesr1, 1.0)
            sel = cst.tile([128, 1], F32)
            nc.vector.memset(sel, 0.0)
            nc.vector.memset(sel[0:1, :], 1.0)
            nc.vector.memset(sel[32:33, :], 1.0)

            qkT = cst.tile([128, 8, N], F32)
            vv = cst.tile([128, HEADS, 8, DH], BF16)
            onorm = cst.tile([128, 4, N], BF16)
            sc_acc = cst.tile([128, N], F32)
            nc.vector.memset(sc_acc, 0.0)

            # ---- QKV q/k part (fp32) ----
            for ct in range(8):
                for ic in range(2):
                    pq = ppA.tile([128, N], F32, tag="dA")
                    for kt in range(4):
                        nc.tensor.matmul(
                            pq[:, ic * 512:(ic + 1) * 512],
                            wq[:, kt, ct * 128:(ct + 1) * 128],
                            xt[:, kt, ic * 512:(ic + 1) * 512],
                            start=(kt == 0), stop=(kt == 3))
                    nc.vector.tensor_copy(qkT[:, ct, ic * 512:(ic + 1) * 512],
                                          pq[:, ic * 512:(ic + 1) * 512])

            # ---- V part (bf16) ----
            for jt in range(8):
                pv = ppB.tile([128, N], F32, tag="dB")
                for kt in range(4):
                    nc.tensor.matmul(
                        pv[:, 0:512],
                        xtb[:, kt, jt * 128:(jt + 1) * 128],
                        wvb[:, kt, :],
                        start=(kt == 0), stop=(kt == 3))
                nc.vector.tensor_copy(
                    vv[:, :, jt, :],
                    pv[:, 0:512].rearrange("p (h d) -> p h d", h=HEADS))

            # ---- attention, head pairs ----
            for pr in range(4):
                hA, hB = 2 * pr, 2 * pr + 1
                oTp = poT.tile([128, N], F32, tag="oT")
                pack = ppk.tile([128, N], F32, tag="pk")
                for jt in range(8):
                    dA = ppA.tile([128, N], F32, tag="dA")
                    dB = ppB.tile([128, N], F32, tag="dB")
                    for ic in range(2):
                        nc.tensor.matmul(
                            dA[:, ic * 512:(ic + 1) * 512],
                            qkT[0:64, 4 + pr, jt * 128:(jt + 1) * 128],
                            qkT[0:64, pr, ic * 512:(ic + 1) * 512],
                            start=True, stop=True, tile_position=(0, 0))
                        nc.tensor.matmul(
                            dB[:, ic * 512:(ic + 1) * 512],
                            qkT[64:128, 4 + pr, jt * 128:(jt + 1) * 128],
                            qkT[64:128, pr, ic * 512:(ic + 1) * 512],
                            start=True, stop=True, tile_position=(64, 0))
                    for hh, dots in ((0, dA), (1, dB)):
                        t = wrk.tile([128, N], F32, tag="t")
                        nc.vector.tensor_tensor(out=t, in0=dots, in1=msk[:, jt, :],
                                                op=AOP.mult)
                        e = wrk.tile([128, N], BF16, tag="e")
                        nc.scalar.activation(out=e, in_=t, func=AFT.Exp, scale=SCALE)
                        ab = wrk.tile([128, N], F32, tag="ab")
                        nc.vector.tensor_scalar(
                            out=ab.bitcast(U32), in0=t.bitcast(U32),
                            scalar1=0x7FFFFFFF, scalar2=None, op0=AOP.bitwise_and)
                        h = hA if hh == 0 else hB
                        first, last = (jt == 0), (jt == 7)
                        for ic in range(2):
                            sl = slice(ic * 512, (ic + 1) * 512)
                            # AV col-packed: head A -> cols 0:64, head B -> 64:128
                            nc.tensor.matmul(
                                oTp[hh * 64:(hh + 1) * 64, sl],
                                vv[:, h, jt, :], e[:, sl],
                                start=first, stop=last,
                                tile_position=(0, hh * 64),
                                skip_group_check=True)
                            # score (fp32) at strips 0/1, Z (bf16) at strips 2/3
                            nc.tensor.matmul(
                                pack[hh * 32:hh * 32 + 1, sl],
                                ones32, ab[:, sl],
                                start=first, stop=last,
                                tile_position=(0, hh * 32),
                                skip_group_check=True)
                            nc.tensor.matmul(
                                pack[64 + hh * 32:64 + hh * 32 + 1, sl],
                                onesbf, e[:, sl],
                                start=first, stop=last,
                                tile_position=(0, 64 + hh * 32),
                                skip_group_check=True)
                # harvest: scores rows {0,32} -> sc_acc; Z rows {64,96}
                nc.vector.tensor_tensor(out=sc_acc[0:33, :], in0=sc_acc[0:33, :],
                                        in1=pack[0:33, :], op=AOP.add)
                zshift = eph.tile([128, 2, N], BF16, tag="zsh")
                nc.scalar.activation(out=zshift[64:65, 0, :], in_=pack[64:65, :],
                                     func=AFT.Copy)
                nc.scalar.activation(out=zshift[96:97, 1, :], in_=pack[96:97, :],
                                     func=AFT.Copy)
                # move Z rows to partition 0 (SBUF->SBUF DMA can shift partitions)
                zrow = eph.tile([1, 2, N], BF16, tag="zrow")
                nc.sync.dma_start(out=zrow[0:1, 0, :], in_=zshift[64:65, 0, :])
                nc.sync.dma_start(out=zrow[0:1, 1, :], in_=zshift[96:97, 1, :])
                # broadcast Z over partitions: rows 0:64 = Z_A, 64:128 = Z_B
                zbc = ppk.tile([128, N], F32, tag="pk")
                for ic in range(2):
                    sl = slice(ic * 512, (ic + 1) * 512)
                    nc.tensor.matmul(zbc[0:64, sl], onesr1[:, 0:64],
                                     zrow[0:1, 0, sl],
                                     start=True, stop=True, tile_position=(0, 0))
                    nc.tensor.matmul(zbc[64:128, sl], onesr1[:, 0:64],
                                     zrow[0:1, 1, sl],
                                     start=True, stop=True, tile_position=(0, 64))
                zr = eph.tile([128, N], F32, tag="zr")
                nc.vector.reciprocal_approx_fast(out=zr, in_=zbc)
                nc.vector.tensor_tensor(out=onorm[:, pr, :], in0=oTp, in1=zr,
                                        op=AOP.mult)

            # ---- output projection (per pair, K=128) ----
            for it in range(8):
                yp = ppA.tile([128, N], F32, tag="dA")
                for pr in range(4):
                    nc.tensor.matmul(
                        yp[:, 0:512],
                        onorm[:, pr, it * 128:(it + 1) * 128],
                        wob[:, pr, :],
                        start=(pr == 0), stop=(pr == 3))
                yt = eph.tile([128, DIM], F32, tag="yt")
                nc.vector.tensor_tensor(out=yt, in0=yp[:, 0:512], in1=bb, op=AOP.add)
                nc.sync.dma_start(out=y_out[it * 128:(it + 1) * 128, :], in_=yt)

            # ---- score: sum the 8 per-head rows, / nnz, * scale ----
            scp = ppB.tile([1, N], F32, tag="dB")
            for ic in range(2):
                sl = slice(ic * 512, (ic + 1) * 512)
                nc.tensor.matmul(scp[0:1, sl], sel, sc_acc[:, sl],
                                 start=True, stop=True)
            nzp = poT.tile([1, N], F32, tag="oT")
            for jt in range(8):
                for ic in range(2):
                    sl = slice(ic * 512, (ic + 1) * 512)
                    nc.tensor.matmul(nzp[0:1, sl], onesbf, msk[:, jt, sl],
                                     start=(jt == 0), stop=(jt == 7))
            scr = eph.tile([1, N], F32, tag="scr")
            rnz = eph.tile([1, N], F32, tag="rnz")
            nc.vector.reciprocal_approx_accurate(out=rnz, in_=nzp, scratch=scr)
            sc_sb = eph.tile([1, N], F32, tag="scs")
            nc.vector.scalar_tensor_tensor(
                out=sc_sb, in0=scp, scalar=SCALE, in1=rnz,
                op0=AOP.mult, op1=AOP.mult)

            # ---- outputs ----
            nc.gpsimd.dma_start(out=sc_out[:, :], in_=sc_sb)
    nc.finalize()
    return nc


def _get_nc():
    if "nc" not in _cache:
        _cache["nc"] = _build()
    return _cache["nc"]


def _run_device(inputs, trace=False):
    x = np.asarray(inputs["x"], np.float32)
    cp_mask = np.asarray(inputs["cp_mask"])
    w_qkv = np.asarray(inputs["w_qkv"], np.float32)
    w_out = np.asarray(inputs["w_out"], np.float32)
    b_out = np.asarray(inputs["b_out"], np.float32)

    bf = mybir.dt.np(BF16)
    maskT = np.ascontiguousarray(cp_mask.T).astype(bf)
    wqk = np.ascontiguousarray(w_qkv[:, :2 * INNER])
    wvbf = np.ascontiguousarray(w_qkv[:, 2 * INNER:]).astype(bf)
    wobf = np.ascontiguousarray(w_out).astype(bf)
    boutr = np.ascontiguousarray(b_out.reshape(1, DIM))

    in_maps = []
    for b in range(B):
        xTb = np.ascontiguousarray(x[b].T)
        in_maps.append({
            "xT": xTb,
            "xTbf": xTb.astype(bf),
            "maskT": maskT,
            "wqk": wqk,
            "wvbf": wvbf,
            "wobf": wobf,
            "bout": boutr,
        })

    nc = _get_nc()
    res = run_bass_kernel_spmd(nc, in_maps, core_ids=list(range(B)), trace=trace)
    y = np.stack([res.results[b]["y"] for b in range(B)])
    score = np.stack([res.results[b]["score"][0] for b in range(B)])
    return y, score, res


def _apply_swap(y, score, patches):
    idx = np.argsort(score, axis=-1, kind="stable")[::-1]
    out = y.copy()
    clone = y
    bi = np.arange(B)
    for i in range(1, patches + 1):
        ti = idx[:, i]
        out[bi, i] = clone[bi, ti]
        out[bi, ti] = clone[:, i]
    return out


def kernel(**inputs):
    patches = int(np.asarray(inputs["patches_in_core_nodes"]))
    y, score, _ = _run_device(inputs, trace=False)
    return _apply_swap(y, score, patches)
